# revision 15
# baseline (speedup 1.0000x reference)
"""Contrastive loss (SimCLR-style) on 8 Trainium2 NeuronCores.

Full inputs in, full output out.  The host normalizes rows (O(N*D), same
category of prep as the baseline's label argmax + per-core feats rolls),
casts to bf16 and ships each core a pre-transposed slab
nfT[c] = nf[rows c*1024 .. c*1024+5119 (mod N)].T  -- the only rows core c
touches.  The device does all O(N^2) work: block matmuls, exp, row sums
and column sums.

Symmetry split: exp(cos/T) is symmetric, so core c computes its 1024
rows against column blocks q=0..4 only (5/8 of the matrix).  Column
sums of every q block are accumulated on the PE with a ones-stationary
matmul and shipped to the host, which routes them to rows of block
(c+q)%8 -- that covers each row's column blocks {b-4..b}.  Direct
row sums (DVE reduce over the bf16 exp tiles) cover blocks b+1..b+3;
block b's own rows are covered by its q=0 column sums (the diagonal
block is exactly symmetric, so colsum == rowsum).  The self-column is
masked with -30 before exp (exp((cos-30)/T) == 0 in fp32).  Positive
pair cosines are O(N*D) and computed on the host.  Host: assemble S,
logsumexp, mean.
"""

from contextlib import ExitStack

import numpy as np

N, D, NCORES = 8192, 128, 8
BLK = N // NCORES            # 1024 rows per core
TPB = BLK // 128             # 8 M-tiles (of 128 rows) per core
NQ = 5                       # column blocks per core (cols 0..5119 rolled)
NCOLS = NQ * BLK             # 5120
TEMP = 0.07
EPS = 1e-8
MASK_SUB = 30.0              # cos - 30 -> exp((cos-30)/T) == 0 in fp32
QCOLS = 1024                 # psum tile columns (2 banks)

_CACHE = {}
LAST_RESULT = None


def _emit(tc, nfT_d, eyeneg_d, rs_out, cs_out, rep=0):
    import concourse.mybir as mybir

    nc = tc.nc
    f32 = mybir.dt.float32
    bf16 = mybir.dt.bfloat16
    AF = mybir.ActivationFunctionType
    AX = mybir.AxisListType.X

    with ExitStack() as ctx:
        singles = ctx.enter_context(tc.tile_pool(name=f"singles{rep}", bufs=1))

        nfT = singles.tile([128, NCOLS], bf16, tag="nfT")
        eyeneg = singles.tile([128, 128], f32, tag="eyeneg")
        ones = singles.tile([128, 128], bf16, tag="ones")
        ebuf = [singles.tile([128, TPB * QCOLS], bf16, tag=f"ebuf{i}",
                             name=f"ebuf{i}_{rep}")
                for i in range(3)]
        css = singles.tile([1, NQ * QCOLS], f32, tag="css")
        rssb = singles.tile([128, 3 * TPB], f32, tag="rssb")
        scr0 = singles.tile([128, 1], f32, tag="scr0")
        scr1 = singles.tile([128, 1], f32, tag="scr1")

        # ---- loads + constants; dummy exp loads the ACT table at t=0 ----
        nc.vector.memset(scr0[:], 0.0)
        nc.scalar.activation(scr1[:], scr0[:], AF.Exp)      # table load only
        nc.vector.memset(ones[:], 1.0)
        nc.sync.dma_start(out=nfT[:, 0:QCOLS], in_=nfT_d[:, 0:QCOLS])
        nc.sync.dma_start(out=eyeneg[:], in_=eyeneg_d)
        for q in range(1, NQ):
            nc.sync.dma_start(out=nfT[:, q * QCOLS:(q + 1) * QCOLS],
                              in_=nfT_d[:, q * QCOLS:(q + 1) * QCOLS])

        # ---- similarity blocks: matmul -> mask(q0) -> exp -> sums ----
        with (
            tc.tile_pool(name=f"mpsum{rep}", bufs=2, space="PSUM") as mpsum,
            tc.tile_pool(name=f"cpsum{rep}", bufs=2, space="PSUM") as cpsum,
        ):
            # PE warm-up: ~24 junk matmuls on the ones tile release the HAM
            # clock gate (~3.4us of activity) while the nfT DMAs land, so the
            # real matmuls run at 2.4 GHz from the start.
            ptw = mpsum.tile([128, QCOLS], f32, tag="mp")
            for _ in range(24):
                nc.tensor.matmul(ptw[:, 0:128], ones[:], ones[:],
                                 start=True, stop=True)

            cstiles = {}

            def emit_cs(q):
                # column sums of exp block q, accumulated across the 8 M-tiles
                # on the PE; all 128 output rows are identical, row 0 ships.
                cs = cpsum.tile([128, QCOLS], f32, tag="cs", name=f"cs{q}_{rep}")
                cstiles[q] = cs
                eb = ebuf[q % 3]
                for m in range(TPB):
                    for h in range(2):
                        nc.tensor.matmul(
                            cs[:, h * 512:(h + 1) * 512], ones[:],
                            eb[:, m * QCOLS + h * 512:m * QCOLS + (h + 1) * 512],
                            start=(m == 0), stop=(m == TPB - 1),
                            skip_group_check=True)

            for q in range(NQ):
                eb = ebuf[q % 3]
                for m in range(TPB):
                    pt = mpsum.tile([128, QCOLS], f32, tag="mp")
                    for h in range(2):
                        nc.tensor.matmul(
                            pt[:, h * 512:(h + 1) * 512],
                            nfT[:, m * 128:(m + 1) * 128],
                            nfT[:, q * QCOLS + h * 512:q * QCOLS + (h + 1) * 512],
                            start=True, stop=True,
                        )
                    if q == 0:
                        # self column of local row m*128+p is m*128+p
                        nc.vector.tensor_add(
                            pt[:, m * 128:(m + 1) * 128],
                            pt[:, m * 128:(m + 1) * 128], eyeneg[:],
                        )
                    nc.scalar.activation(
                        eb[:, m * QCOLS:(m + 1) * QCOLS], pt[:],
                        AF.Exp, scale=1.0 / TEMP,
                    )
                    if 1 <= q <= 3:
                        # direct row sum of this tile; per-m so the DVE work
                        # pipelines inside the ACT window instead of piling
                        # into a serial tail.
                        nc.vector.reduce_sum(
                            out=rssb[:, (q - 1) * TPB + m:(q - 1) * TPB + m + 1],
                            in_=eb[:, m * QCOLS:(m + 1) * QCOLS], axis=AX)
                # delay q's colsum matmuls by one q so the PE FIFO never
                # blocks the next q's direct matmuls on ACT output.
                if q >= 1:
                    emit_cs(q - 1)
                if q >= 2:
                    qq = q - 2   # cs(q-2) fully accumulated; ship row 0
                    nc.vector.tensor_copy(
                        css[0:1, qq * QCOLS:(qq + 1) * QCOLS],
                        cstiles.pop(qq)[0:1, :])
            emit_cs(NQ - 1)
            for qq in (NQ - 2, NQ - 1):
                nc.vector.tensor_copy(
                    css[0:1, qq * QCOLS:(qq + 1) * QCOLS],
                    cstiles.pop(qq)[0:1, :])

        nc.sync.dma_start(out=rs_out, in_=rssb[:])
        nc.sync.dma_start(out=cs_out, in_=css[:])


def _build_nc(repeats=1):
    import concourse.tile as tile
    import concourse.mybir as mybir
    from concourse import bacc

    f32 = mybir.dt.float32
    bf16 = mybir.dt.bfloat16
    nc = bacc.Bacc(
        "TRN2", target_bir_lowering=False, debug=False,
        enable_asserts=False, num_devices=NCORES,
    )
    nfT_h = nc.dram_tensor("nfT", [128, NCOLS], bf16, kind="ExternalInput")
    en_h = nc.dram_tensor("eyeneg", [128, 128], f32, kind="ExternalInput")
    rs_h = nc.dram_tensor("rs_out", [128, 3 * TPB], f32, kind="ExternalOutput")
    cs_h = nc.dram_tensor("cs_out", [1, NQ * QCOLS], f32, kind="ExternalOutput")

    with tile.TileContext(nc, trace_sim=False) as tc:
        for rep in range(repeats):
            _emit(tc, nfT_h.ap(), en_h.ap(), rs_h.ap(), cs_h.ap(), rep=rep)
    nc.compile()
    return nc


def get_nc(repeats=1):
    key = ("nc", repeats)
    if key not in _CACHE:
        _CACHE[key] = _build_nc(repeats)
    return _CACHE[key]


def make_in_maps(feats, label):
    """Host prep: normalize rows (fp32, eps-clamped like F.cosine_similarity),
    bf16-cast, and build each core's transposed slab of the 5120 rows it
    needs.  Also returns the positive-pair cosines (O(N*D), host)."""
    import ml_dtypes

    feats = np.ascontiguousarray(np.asarray(feats, dtype=np.float32))
    label = np.asarray(label)
    norms = np.sqrt(np.sum(feats.astype(np.float64) ** 2, axis=1))
    nf64 = feats / np.maximum(norms, EPS)[:, None]
    nfb = nf64.astype(ml_dtypes.bfloat16)

    pos_idx = np.argmax(label, axis=1)
    pos = np.einsum("nd,nd->n", nf64, nf64[pos_idx])

    eyeneg = (-MASK_SUB * np.eye(128)).astype(np.float32)
    in_maps = []
    for c in range(NCORES):
        rows = (np.arange(NCOLS) + c * BLK) % N
        nfT = np.ascontiguousarray(nfb[rows].T)          # [128, 5120] bf16
        in_maps.append({"nfT": nfT, "eyeneg": eyeneg})
    return in_maps, pos


def finish(results, pos):
    """Host epilogue: route row/column partial sums, logsumexp, mean."""
    S = np.zeros(N, dtype=np.float64)
    for c in range(NCORES):
        rs = results[c]["rs_out"].astype(np.float64)     # [128, 3*TPB]
        # rs[p, (q-1)*8 + m] = block-(c+q) partial row sum of row m*128+p
        blk = rs.reshape(128, 3, TPB).sum(axis=1)        # [p, m]
        S[c * BLK:(c + 1) * BLK] += blk.T.reshape(-1)
        cs = results[c]["cs_out"].astype(np.float64).reshape(NQ, QCOLS)
        for q in range(NQ):
            tgt = ((c + q) % NCORES) * BLK
            S[tgt:tgt + BLK] += cs[q]
    lse = np.log(S)
    loss = (lse - pos / TEMP).mean()
    return np.array(loss, dtype=np.float32)


def kernel(feats, label, _trace=False, _repeats=1):
    global LAST_RESULT
    from concourse.bass_utils import run_bass_kernel_spmd

    nc = get_nc(_repeats)
    in_maps, pos = make_in_maps(feats, label)
    res = run_bass_kernel_spmd(nc, in_maps, list(range(NCORES)), trace=_trace)
    LAST_RESULT = res
    return finish(res.results, pos)
